# revision 24
# baseline (speedup 1.0000x reference)
"""Trainium2 Bass kernel for nn_Jacobi_layer: 20 Jacobi sweeps over 32
independent 512x512 grids (5-point stencil, reflect padding, Dirichlet mask,
source term f = COF*layout with COF ~ 1e-8 -- numerically negligible,
dropped). Sharding: pure data parallelism -- 4 samples per core across 8
NeuronCores. State is bf16 (rel err ~9.6e-3 over 20 sweeps, gate 2e-2).

Design (v7): 4 row-chunks of 128 with STAGED corner rows.

Grid rows split into 4 chunks of 128 (rows = partitions). Per block
(sample s, iteration t):
  - TensorE : per chunk, a tridiagonal matmul (vertical stencil, 0.25
              weights + reflect rows baked into the LHS), a K=2 "corner"
              matmul adding the cross-chunk boundary rows from a staging
              tile S, and a 0.25*I identity matmul accumulating
              T = x_left + x_right. 12 x N=512 matmuls.
  - VectorE : T for block b+2 (one shifted tensor_add over the
              ghost-column-padded tile). Nothing else -- fully decoupled.
  - ScalarE : psum->bf16 evacuation, two ACTIVATEs ([c0,c1], [c2,c3]),
              each emitted right after the matmul that completes its
              region.
  - GpSimd  : Dirichlet mask (one strided memset: col 0 of chunks 1,2 =
              rows 128..383) + ghost-column refresh.
  - DMA     : 2 staging DMAs re-filling S[s] from x_new's boundary rows
              (consumed by the corner matmuls one iteration later).

PSUM: two pool tiles of 4 banks each (bufs=2) -- every cross-engine
dependency has >= 1.5 blocks of float, so no engine ever stalls the PE
(the v2-v6 5-chunk/8-bank-ring variants all convoyed at ~290-310us because
the ring's WAR tracking serialized each block's matmuls behind the previous
block's evacuation).

Staging tile S[s] is [2, 2048]: row k0 = "row after chunk c" (128, 256,
384, ZERO), row k1 = "row before chunk c" (ZERO, 127, 255, 383); a single
shared LHS [2,128] with 0.25 at (k0 -> p127) and (k1 -> p0) serves all
four chunks (the zero blocks cover the reflect edges, which the tridiag
LHS handles instead).
"""
import sys
import numpy as np

if "/opt/trn_rl_repo" not in sys.path:
    sys.path.insert(0, "/opt/trn_rl_repo")

from contextlib import ExitStack

import ml_dtypes
import concourse.bass as bass
import concourse.bacc as bacc
import concourse.tile as tile
import concourse.mybir as mybir
from concourse.bass_utils import run_bass_kernel_spmd

NX = 512
P = 128
NCHUNK = 4
PW = NX + 4        # padded chunk width: [pad, ghostL, 512 data, ghostR, pad]
DOFF = 2           # data offset inside a chunk (even -> 4B-aligned bf16)
BATCH = 32
NCORES = 8
SPC = BATCH // NCORES   # samples per core = 4
WP = NCHUNK * PW        # 2064 padded free elems per x tile
WPA = 2304              # x tile free size (512B-multiple base stride)
W = NCHUNK * NX         # 2048 compact free elems (T tile / psum per block)

BF16 = mybir.dt.bfloat16
F32 = mybir.dt.float32
NP_BF16 = ml_dtypes.bfloat16


def _build_consts() -> np.ndarray:
    """[128, 512] bf16: A_top | A_mid | A_bot | 0.25*I  (all [K, M]).
    Corner LHS is tiny and lives in A_top's unused corner: rows 0..1 of
    a separate 128-col block would collide with A_top's band, so it gets
    its own encoding below (cor): entries at [k0, 127] and [k1, 0]."""
    c = np.zeros((P, 512 + 128), dtype=np.float32)
    top = c[:, 0:128]
    for m in range(1, 128):
        top[m - 1, m] = 0.25
        if m < 127:
            top[m + 1, m] = 0.25
    top[1, 0] = 0.5           # reflect: row 0 vertical sum = 2*x[1]
    mid = c[:, 128:256]
    for m in range(128):
        if m > 0:
            mid[m - 1, m] = 0.25
        if m < 127:
            mid[m + 1, m] = 0.25
    bot = c[:, 256:384]
    for m in range(128):
        if m > 0:
            bot[m - 1, m] = 0.25
        if m < 127:
            bot[m + 1, m] = 0.25
    bot[126, 127] = 0.5       # reflect: row 511 vertical sum = 2*x[510]
    c[:, 384:512] = 0.25 * np.eye(P, dtype=np.float32)
    cor = c[:, 512:640]
    cor[0, 127] = 0.25        # S row k0 = "row after chunk" -> out p127
    cor[1, 0] = 0.25          # S row k1 = "row before chunk" -> out p0
    return c.astype(NP_BF16)


def _build(n_iter: int):
    nc = bacc.Bacc("TRN2", target_bir_lowering=False, debug=False,
                   num_devices=NCORES)

    heat_d = nc.dram_tensor("heat", [SPC, NCHUNK, P, PW], BF16,
                            kind="ExternalInput")
    cst_d = nc.dram_tensor("consts", [P, 640], BF16, kind="ExternalInput")
    out_d = nc.dram_tensor("out", [SPC, NCHUNK, P, NX], BF16,
                           kind="ExternalOutput")

    with tile.TileContext(nc) as tc:
        with ExitStack() as ctx:
            state = ctx.enter_context(tc.tile_pool(name="state", bufs=1))
            tpool = ctx.enter_context(tc.tile_pool(name="tpool", bufs=4))
            ppool = ctx.enter_context(
                tc.tile_pool(name="ppool", bufs=2, space=bass.MemorySpace.PSUM))

            cst = state.tile([P, 640], BF16, tag="cst")
            nc.sync.dma_start(cst[:], cst_d.ap())
            lhs_top = cst[:, 0:128]
            lhs_mid = cst[:, 128:256]
            lhs_bot = cst[:, 256:384]
            lhs_idn = cst[:, 384:512]
            lhs_cor = cst[0:2, 512:640]

            xa, xb, stg = [], [], []
            dma_eng = [nc.sync, nc.scalar, nc.gpsimd, nc.sync]
            for s in range(SPC):
                x0 = state.tile([P, WPA], BF16, tag=f"xa{s}", name=f"xa{s}")
                dma_eng[s].dma_start(
                    x0[:, 0:WP].rearrange("p (c j) -> p c j", c=NCHUNK),
                    heat_d.ap()[s].rearrange("c p j -> p c j"))
                xa.append(x0)
                xb.append(state.tile([P, WPA], BF16, tag=f"xb{s}",
                                     name=f"xb{s}"))
                st = state.tile([P, W], BF16, tag=f"stg{s}", name=f"stg{s}")
                # zero the unused corner blocks once (k0 chunk3, k1 chunk0)
                nc.gpsimd.memset(st[0:2, :], 0.0)
                # initial staging from x0 (after its load DMA)
                nc.sync.dma_start(
                    st[0:1, 0:3 * NX].rearrange("p (c j) -> p c j", c=3),
                    x0[0:1, PW:WP].rearrange(
                        "p (c j) -> p c j", c=3)[:, :, DOFF:DOFF + NX])
                nc.sync.dma_start(
                    st[1:2, NX:W].rearrange("p (c j) -> p c j", c=3),
                    x0[127:128, 0:3 * PW].rearrange(
                        "p (c j) -> p c j", c=3)[:, :, DOFF:DOFF + NX])
                stg.append(st)

            def x3(x):
                return x[:, 0:WP].rearrange("p (c j) -> p c j", c=NCHUNK)

            def make_T(xsrc):
                T = tpool.tile([P, W], BF16, tag="T", name="T")
                t3 = T.rearrange("p (c j) -> p c j", c=NCHUNK)
                xs = x3(xsrc)
                nc.vector.tensor_add(
                    t3[:, :, 0:NX], xs[:, :, 1:NX + 1], xs[:, :, 3:NX + 3])
                return T

            # PE warm-up on the consts tile covers the input DMAs.
            warm = ppool.tile([P, W], F32, tag="psum", name="warm")
            for _k in range(22):
                nc.tensor.matmul(warm[:, 0:NX], lhs_idn, cst[:, 0:NX],
                                 start=True, stop=True)

            cur, nxt = xa, xb
            nblocks = n_iter * SPC
            T_queue = [make_T(xa[0]), make_T(xa[1])]
            for t in range(n_iter):
                last_t = t == n_iter - 1
                for s in range(SPC):
                    b = t * SPC + s
                    x, xn, st = cur[s], nxt[s], stg[s]
                    xv = x3(x)
                    xn3 = x3(xn)

                    # --- DVE: T for block b+2, emitted first ---
                    nb = b + 2
                    if nb < nblocks:
                        t2, s2 = divmod(nb, SPC)
                        T_queue.append(
                            make_T(cur[s2] if t2 == t else nxt[s2]))

                    psum = ppool.tile([P, W], F32, tag="psum", name="psum")
                    prev_T = T_queue.pop(0)

                    def psl(c):
                        return psum[:, c * NX:(c + 1) * NX]

                    def chunk_mms(c, lhs):
                        nc.tensor.matmul(psl(c), lhs,
                                         xv[:, c, DOFF:DOFF + NX],
                                         start=True, stop=False)
                        nc.tensor.matmul(psl(c), lhs_cor,
                                         st[0:2, c * NX:(c + 1) * NX],
                                         start=False, stop=False)
                        nc.tensor.matmul(psl(c), lhs_idn,
                                         prev_T[:, c * NX:(c + 1) * NX],
                                         start=False, stop=True)

                    chunk_mms(0, lhs_top)
                    chunk_mms(1, lhs_mid)
                    nc.scalar.copy(
                        xn3[:, 0:2, DOFF:DOFF + NX],
                        psum[:, 0:2 * NX].rearrange("p (c j) -> p c j", c=2))
                    chunk_mms(2, lhs_mid)
                    chunk_mms(3, lhs_bot)
                    nc.scalar.copy(
                        xn3[:, 2:4, DOFF:DOFF + NX],
                        psum[:, 2 * NX:W].rearrange("p (c j) -> p c j", c=2))

                    # --- GpSimd: Dirichlet mask (zero col 0 of rows
                    # 128..383 = chunks 1,2 entirely) + ghost-col refresh
                    nc.gpsimd.memset(
                        xn[:, PW + DOFF:2 * PW + DOFF + 1:PW], 0.0)
                    if not last_t:
                        nc.gpsimd.tensor_copy(
                            xn3[:, :, 1:2], xn3[:, :, 3:4])
                        nc.gpsimd.tensor_copy(
                            xn3[:, :, PW - 2:PW - 1],
                            xn3[:, :, PW - 4:PW - 3])
                        # --- staging DMAs: S[s] <- x_new boundary rows
                        # (consumed by corner matmuls next iteration)
                        nc.sync.dma_start(
                            st[0:1, 0:3 * NX].rearrange(
                                "p (c j) -> p c j", c=3),
                            xn[0:1, PW:WP].rearrange(
                                "p (c j) -> p c j", c=3)[:, :,
                                                         DOFF:DOFF + NX])
                        nc.sync.dma_start(
                            st[1:2, NX:W].rearrange("p (c j) -> p c j", c=3),
                            xn[127:128, 0:3 * PW].rearrange(
                                "p (c j) -> p c j", c=3)[:, :,
                                                         DOFF:DOFF + NX])
                cur, nxt = nxt, cur

            for s in range(SPC):
                dma_eng[s].dma_start(
                    out_d.ap()[s].rearrange("c p j -> p c j"),
                    x3(cur[s])[:, :, DOFF:DOFF + NX])

    nc.compile()
    return nc


_CACHE: dict = {}


def _get_nc(n_iter: int):
    if n_iter not in _CACHE:
        _CACHE[n_iter] = _build(n_iter)
    return _CACHE[n_iter]


def _prep_heat(heat: np.ndarray) -> np.ndarray:
    """[B,512,512] fp32 -> [B,4,128,516] bf16 padded, masked, with ghost
    columns."""
    b = heat.shape[0]
    h = heat.copy()
    h[:, 128:384, 0] = 0.0    # x0 = heat * G
    hc = h.reshape(b, NCHUNK, P, NX)
    hp = np.zeros((b, NCHUNK, P, PW), dtype=np.float32)
    hp[..., DOFF:NX + DOFF] = hc
    hp[..., DOFF - 1] = hc[..., 1]         # ghost-left = col 1
    hp[..., NX + DOFF] = hc[..., NX - 2]   # ghost-right = col 510
    return hp.astype(NP_BF16)


def run(layout, heat, n_iter, trace=False):
    n_iter = int(n_iter)
    heat = np.ascontiguousarray(np.asarray(heat, dtype=np.float32)
                                .reshape(BATCH, NX, NX))
    hp = _prep_heat(heat)
    consts = _build_consts()
    nc = _get_nc(n_iter)
    in_maps = []
    for c in range(NCORES):
        sl = slice(c * SPC, (c + 1) * SPC)
        in_maps.append({"heat": hp[sl], "consts": consts})
    res = run_bass_kernel_spmd(nc, in_maps, list(range(NCORES)), trace=trace)
    out = np.concatenate(
        [res.results[c]["out"].reshape(SPC, NX, NX) for c in range(NCORES)],
        axis=0)
    return out.astype(np.float32).reshape(BATCH, 1, NX, NX), res


def kernel(layout, heat, n_iter):
    out, _ = run(layout, heat, n_iter)
    return out
